# revision 1
# baseline (speedup 1.0000x reference)
"""Trainium2 Bass kernel for the CubeSimulator problem.

Reference computation (shapes): rotate (96,96,96) grids, build a per-voxel
line-of-sight velocity u and intensity I = exp(L), then a Gaussian-KDE cube
cube[i,j,v] = norm * sum_z exp(-(vel_v - u)^2/sig^2) * I, followed by a
"trilinear" downsample (96,96,64) -> (32,64,64).

Key exact simplifications (validated against the reference in fp32,
rel err ~6e-6):
 - downsample axis0 (96->32, scale 3): output coords land exactly on
   integers 3k+1, so it is a pure row selection -> only 32 of 96 i-rows
   are ever needed (3x less KDE work).
 - downsample axis2 (64->64) is exactly the identity.
 - downsample axis1 (96->64) is an exact 2-tap stencil with weights
   0.75/0.25 (even) / 0.25/0.75 (odd), applied as one TensorE matmul over
   the j partition axis.
 - exp(L - (vel_v-u)^2/sig^2) = exp(A + vel_v*B + c_v) with
   A = L + ln(norm) - u^2/sig^2, B = 2u/sig^2, c_v = -vel_v^2/sig^2;
   A and B are precomputed per voxel.
 - tanh(r/2)/r = (e^r - 1)/(r (e^r + 1)) evaluated with a single
   reciprocal; r = exp(0.5 ln(max(q,1e-35))) keeps every activation in
   the natural_log_exp_and_others table set (one ACT table load) and
   avoids the loose-tolerance Sqrt.

Per velocity bin, two engine-balanced paths (split tuned on the
instruction cost model):
 - affine path: VectorE tensor_scalar (B*vv + c_v), V/G tensor_add (+A),
   with KDE_VB bins batched into one wide ScalarE Exp.
 - factored path: exp(A + vv*B + c_v) = exp(A) * exp(vv*B + c_v) -- one
   ScalarE Exp (scale=vv immediate, bias=c_v per-partition AP) and one
   V/G multiply by P0 = exp(A).
The z-reduction is a per-(bin, i-row) TensorE matmul with the exp tile as
the stationary operand and a ones-vector moving, accumulating the cube as
[j=96 partitions, (i,v)] in PSUM, which makes the j-downsample a single
stationary-W matmul.

Sharding: the 32 needed i-rows are split 4-per-core across 8 cores (pure
data parallel over pixels); each core's device layout is [z=96 partitions,
pixels=4*96=384 free].  Runtime scalars (rotation trig, sigma, the 64
velocity values) are baked into the instruction stream as immediates since
the kernel is compiled per call.
"""

import math

import numpy as np

import concourse.bacc as bacc
import concourse.bass as bass
import concourse.mybir as mybir
import concourse.tile as tile
from concourse.bass_utils import run_bass_kernel_spmd

G = 96            # up_gal grid size
NV = 64           # velocity bins
N_CORES = 8
OUT_I = 32        # downsampled i rows (= VEL_RES in the reference's axis naming)
ROWS_PER_CORE = OUT_I // N_CORES   # 4
PX = ROWS_PER_CORE * G             # 384 pixels per core
OUT_J = 64

F32 = mybir.dt.float32
AF = mybir.ActivationFunctionType
OP = mybir.AluOpType

LAST_EXEC_NS = None  # filled in when run with BASS_TRACE=1

# tuning knobs (validated via TimelineSim sweeps)
KDE_VB = 8                   # velocity bins per group
KDE_FACT_SLOTS = (1, 3, 5, 7)  # slots per group using the factored path
KDE_NGC = 0.25               # fraction of affine-path adds routed to GpSimd
KDE_FACT_G = 0.75            # fraction of factored-path mults routed to GpSimd
ABLATE = set()         # {'mm','tt','ts','exp'} - sim-only ablation switches


def _build_program(ci, si, cr, sr, sig2, lnnorm, vel, fact_slots=None):
    if fact_slots is None:
        fact_slots = KDE_FACT_SLOTS
    nc = bacc.Bacc("TRN2")

    xs = nc.dram_tensor("xs", [G, PX], F32, kind="ExternalInput")
    ys = nc.dram_tensor("ys", [G, PX], F32, kind="ExternalInput")
    zs = nc.dram_tensor("zs", [G, PX], F32, kind="ExternalInput")
    # per-velocity-bin exp biases c_v = -vel_v^2/sig^2 (replicated across
    # partitions; used as per-partition bias APs on factored-path Exps)
    bc = nc.dram_tensor("bc", [128, NV], F32, kind="ExternalInput")
    # j-downsample stencil matrix (96 -> 64, 2 taps per output)
    wj = nc.dram_tensor("wj", [G, OUT_J], F32, kind="ExternalInput")
    out = nc.dram_tensor("out", [OUT_J, ROWS_PER_CORE * NV], F32,
                         kind="ExternalOutput")

    with tile.TileContext(nc) as tc:
        with (
            tc.tile_pool(name="io", bufs=1) as io,
            tc.tile_pool(name="prep", bufs=1) as prep,
            tc.tile_pool(name="kde", bufs=2) as kde,
            tc.tile_pool(name="psum", bufs=1, space="PSUM") as psum,
        ):
            xt = io.tile([G, PX], F32, tag="xt")
            yt = io.tile([G, PX], F32, tag="yt")
            zt = io.tile([G, PX], F32, tag="zt")
            nc.sync.dma_start(out=xt[:], in_=xs[:])
            nc.sync.dma_start(out=yt[:], in_=ys[:])
            nc.sync.dma_start(out=zt[:], in_=zs[:])
            wjt = io.tile([G, OUT_J], F32, tag="wjt")
            nc.sync.dma_start(out=wjt[:], in_=wj[:])
            bct = io.tile([128, NV], F32, tag="bct")
            nc.sync.dma_start(out=bct[:], in_=bc[:])

            def vtile(name):
                return prep.tile([G, PX], F32, tag=name, name=name)

            # Prep. Only tensor_scalar / tensor_tensor / activation are used
            # -- the S2S2D2_STT (scalar_tensor_tensor) ISA struct has a
            # single sync-wait slot and cannot be scheduled where Tile needs
            # multiple waits.
            # Rotated coordinates (R = Rx(inc) @ Rz(rot)); the rx/ry legs run
            # on VectorE (critical path), the rz/intensity leg on GpSimd.
            xa, ya, rx = vtile("xa"), vtile("ya"), vtile("rx")
            xb, yb, t3 = vtile("xb"), vtile("yb"), vtile("t3")
            za, ry = vtile("za"), vtile("ry")
            nc.vector.tensor_scalar_mul(xa[:], xt[:], cr)
            nc.vector.tensor_scalar_mul(ya[:], yt[:], -sr)
            nc.vector.tensor_add(rx[:], xa[:], ya[:])
            nc.vector.tensor_scalar_mul(xb[:], xt[:], ci * sr)
            nc.vector.tensor_scalar_mul(yb[:], yt[:], ci * cr)
            nc.vector.tensor_add(t3[:], xb[:], yb[:])
            nc.vector.tensor_scalar_mul(za[:], zt[:], -si)
            nc.vector.tensor_add(ry[:], t3[:], za[:])
            xc, yc, t5 = vtile("xc"), vtile("yc"), vtile("t5")
            zb, rz = vtile("zb"), vtile("rz")
            nc.gpsimd.tensor_scalar_mul(xc[:], xt[:], si * sr)
            nc.gpsimd.tensor_scalar_mul(yc[:], yt[:], si * cr)
            nc.gpsimd.tensor_add(t5[:], xc[:], yc[:])
            nc.gpsimd.tensor_scalar_mul(zb[:], zt[:], ci)
            nc.gpsimd.tensor_add(rz[:], t5[:], zb[:])

            # in-plane radius r via exp(0.5*ln(q)) -- avoids the loose-
            # tolerance Sqrt activation.  q is clamped away from 0 once so
            # every division below is finite (r >= 3e-18).
            sqx, sqy, q, qs = vtile("sqx"), vtile("sqy"), vtile("q"), vtile("qs")
            lnq, r = vtile("lnq"), vtile("r")
            nc.scalar.activation(sqx[:], rx[:], AF.Square)
            nc.vector.tensor_mul(sqy[:], ry[:], ry[:])
            nc.vector.tensor_add(q[:], sqy[:], sqx[:])
            nc.vector.tensor_scalar_max(qs[:], q[:], 1e-35)
            nc.scalar.activation(lnq[:], qs[:], AF.Ln)
            nc.scalar.activation(r[:], lnq[:], AF.Exp, scale=0.5)

            # u0 = rx*tanh(r/2)/r computed as rx*(e^r-1) / (r*(e^r+1)) --
            # one reciprocal, and every activation stays in the
            # natural_log_exp_and_others table set (single table load).
            # The -200*si amplitude folds into the s1/Bt scales below.
            er, ed = vtile("er"), vtile("ed")
            den, rec, num = vtile("den"), vtile("rec"), vtile("num")
            t1, u0 = vtile("t1"), vtile("u0")
            nc.scalar.activation(er[:], r[:], AF.Exp)
            nc.vector.tensor_scalar_add(ed[:], er[:], 1.0)
            nc.vector.tensor_mul(den[:], ed[:], r[:])
            nc.vector.reciprocal(rec[:], den[:])
            nc.vector.tensor_scalar_add(num[:], er[:], -1.0)
            nc.vector.tensor_mul(t1[:], rx[:], num[:])
            nc.vector.tensor_mul(u0[:], t1[:], rec[:])

            # A = L + lnnorm - (u/sig)^2 ; L = -r/3 - 2|rz| ; B = 2u/sig^2
            az, azs, rterm, Lt = (vtile("az"), vtile("azs"), vtile("rterm"),
                                  vtile("Lt"))
            s1, ssq, At, Bt, P0t = (vtile("s1"), vtile("ssq"), vtile("At"),
                                    vtile("Bt"), vtile("P0t"))
            nc.scalar.activation(az[:], rz[:], AF.Abs)
            nc.gpsimd.tensor_scalar_mul(azs[:], az[:], -2.0)
            nc.gpsimd.tensor_scalar(rterm[:], r[:], -1.0 / 3.0, lnnorm,
                                    OP.mult, OP.add)
            nc.gpsimd.tensor_add(Lt[:], azs[:], rterm[:])
            usc = -200.0 * si
            nc.vector.tensor_scalar_mul(s1[:], u0[:], usc / math.sqrt(sig2))
            nc.scalar.activation(ssq[:], s1[:], AF.Square)
            nc.vector.tensor_sub(At[:], Lt[:], ssq[:])
            nc.vector.tensor_scalar_mul(Bt[:], u0[:], usc * 2.0 / sig2)
            nc.scalar.activation(P0t[:], At[:], AF.Exp)

            ones = io.tile([G, 1], F32, tag="ones")
            nc.vector.memset(ones[:], 1.0)

            # cube[j, i*NV + v] = sum_z exp-term   (j on partitions)
            cube = psum.tile([G, ROWS_PER_CORE * NV], F32)

            # Two paths per velocity bin, mixed to balance engines:
            #  - affine path: arg = (B*vv + c_v) + A  (VectorE ts + V/G tt),
            #    VB-batched into one wide ScalarE Exp.
            #  - factored path: exp(A + vv*B + c_v) = P0 * exp(vv*B + c_v)
            #    (one ScalarE Exp with scale/bias immediates + one V/G mult;
            #    exact to fp32 rounding since both factors are exp outputs).
            VB = KDE_VB
            for g in range(NV // VB):
                bins = list(range(g * VB, (g + 1) * VB))
                cbins = [b for b in bins if (b % VB) not in fact_slots]
                fbins = [b for b in bins if (b % VB) in fact_slots]
                srcs = {}
                # factored-path bins first: their Exps depend only on Bt, so
                # ScalarE starts each group without stalling on the affine
                # arg builds (Tile priority follows emission order)
                nfb = len(fbins)
                for k, iv in enumerate(fbins):
                    vv = float(vel[iv])
                    e1 = kde.tile([G, PX], F32, tag="e1", bufs=4)
                    nc.scalar.activation(e1[:], Bt[:], AF.Exp, scale=vv,
                                         bias=bct[0:G, iv:iv + 1])
                    m1 = kde.tile([G, PX], F32, tag="m1", bufs=4)
                    eng = nc.gpsimd if k < KDE_FACT_G * nfb else nc.vector
                    eng.tensor_mul(m1[:], e1[:], P0t[:])
                    srcs[iv] = (m1, 0)
                ncb = len(cbins)
                if ncb:
                    argw = kde.tile([G, ncb * PX], F32, tag="argw")
                    tmpw = kde.tile([G, ncb * PX], F32, tag="tmpw")
                    for k, iv in enumerate(cbins):
                        vv = float(vel[iv])
                        cv = -vv * vv / sig2
                        sl = slice(k * PX, (k + 1) * PX)
                        nc.vector.tensor_scalar(tmpw[:, sl], Bt[:], vv, cv,
                                                OP.mult, OP.add)
                        eng = nc.gpsimd if k < KDE_NGC * ncb else nc.vector
                        eng.tensor_add(argw[:, sl], tmpw[:, sl], At[:])
                    exw = kde.tile([G, ncb * PX], F32, tag="exw")
                    nc.scalar.activation(exw[:], argw[:], AF.Exp)
                    for k, iv in enumerate(cbins):
                        srcs[iv] = (exw, k * PX)
                # reduce over z (partitions) one i-row at a time: E-slice is
                # the stationary operand, a ones-vector the moving one.
                for iv in bins:
                    if 'mm' in ABLATE:
                        break
                    t, off0 = srcs[iv]
                    for ii in range(ROWS_PER_CORE):
                        col = ii * NV + iv
                        off = off0 + ii * G
                        nc.tensor.matmul(cube[:, col:col + 1],
                                         t[:, off:off + G], ones[:],
                                         start=True, stop=True)

            # j-downsample over the partition axis: out2[jj, (i,v)]
            cube_sb = io.tile([G, ROWS_PER_CORE * NV], F32, tag="cube_sb")
            nc.vector.tensor_copy(cube_sb[:], cube[:])
            out_ps = psum.tile([OUT_J, ROWS_PER_CORE * NV], F32)
            nc.tensor.matmul(out_ps[:], wjt[:], cube_sb[:],
                             start=True, stop=True)
            out_sb = io.tile([OUT_J, ROWS_PER_CORE * NV], F32, tag="out_sb")
            nc.vector.tensor_copy(out_sb[:], out_ps[:])
            nc.sync.dma_start(out=out[:], in_=out_sb[:])

    return nc


def kernel(**inputs):
    inc = float(np.asarray(inputs["inclination"]).reshape(-1)[0])
    rot = float(np.asarray(inputs["sky_rot"]).reshape(-1)[0])
    lb = float(np.asarray(inputs["line_broadening"]).reshape(-1)[0])
    vel = np.asarray(inputs["velocity_grid"], np.float32).reshape(-1)
    X = np.asarray(inputs["Xgrid"], np.float32)
    Y = np.asarray(inputs["Ygrid"], np.float32)
    Z = np.asarray(inputs["Zgrid"], np.float32)

    ci, si = math.cos(inc), math.sin(inc)
    cr, sr = math.cos(rot), math.sin(rot)
    sig2 = float(np.float32(lb) * np.float32(lb))
    if not (sig2 > 0.0) or not math.isfinite(sig2):
        sig2 = 1e-30  # degenerate sigma: reference output is ~0/NaN anyway
    lnnorm = float(-0.5 * math.log(2.0 * math.pi * sig2))

    # The factored path computes exp(vv*B + c_v) whose argument is bounded by
    # u_max^2/sig^2 (u_max = 200*|sin(inc)| rigorously bounds |u|).  If that
    # could overflow fp32, fall back to the always-safe affine path (its
    # fused exponent is <= ln(norm)).
    umax2 = (200.0 * abs(si)) ** 2
    fact_slots = KDE_FACT_SLOTS if umax2 / sig2 <= 80.0 else ()
    nc = _build_program(ci, si, cr, sr, sig2, lnnorm, vel, fact_slots)
    nc.finalize()

    bcv = np.ascontiguousarray(
        np.tile((-(vel.astype(np.float64) ** 2) / sig2).astype(np.float32),
                (128, 1)))
    wjv = np.zeros((G, OUT_J), np.float32)
    for m in range(OUT_J // 2):
        wjv[3 * m, 2 * m] = 0.75
        wjv[3 * m + 1, 2 * m] = 0.25
        wjv[3 * m + 1, 2 * m + 1] = 0.25
        wjv[3 * m + 2, 2 * m + 1] = 0.75

    in_maps = []
    for c in range(N_CORES):
        rows = [3 * k + 1 for k in range(ROWS_PER_CORE * c,
                                         ROWS_PER_CORE * (c + 1))]
        def shard(a):
            s = a[rows]                        # (4, 96, 96) = (i, j, z)
            s = s.transpose(2, 0, 1).reshape(G, PX)   # [z, i*96+j]
            return np.ascontiguousarray(s)
        in_maps.append({"xs": shard(X), "ys": shard(Y), "zs": shard(Z),
                        "bc": bcv, "wj": wjv})

    res = run_bass_kernel_spmd(nc, in_maps, core_ids=list(range(N_CORES)))
    global LAST_EXEC_NS
    LAST_EXEC_NS = res.exec_time_ns

    parts = []
    for c in range(N_CORES):
        o = res.results[c]["out"]              # (64, 256) = [jj, i*64+v]
        parts.append(o.reshape(OUT_J, ROWS_PER_CORE, NV).transpose(1, 0, 2))
    return np.concatenate(parts, axis=0).astype(np.float32)  # (32, 64, 64)



# revision 18
# speedup vs baseline: 1.6661x; 1.6661x over previous
"""Trainium2 Bass kernel for the CubeSimulator problem (v2).

Reference: rotate (96,96,96) grids, per-voxel line-of-sight velocity u and
intensity I, Gaussian-KDE cube[i,j,v] = norm * sum_z exp(-(vel_v-u)^2/sig^2)*I,
then trilinear downsample (96,96,64) -> (32,64,64).

Exact structural simplifications (from the v1 baseline, validated):
 - axis0 96->32 downsample is a pure row selection (rows 3k+1);
 - axis2 64->64 downsample is the identity;
 - axis1 96->64 is an exact 2-tap stencil 0.75/0.25 (even) 0.25/0.75 (odd).

New in v2 (all validated numerically against the reference, ~1.3e-3 rel):
 - Gaussian-frame factorization: K_sig(vv-u) ~= sum_k M[k,v] * K_sig'(w_k-u)
   with sig'^2 = sig^2/2, on a coarse w-grid (NK ~= 24 << 64 bins).  M is a
   tiny host-side least-squares fit -- it only depends on runtime scalars
   (sigma, vel grid), like the baseline's bc/wj constants.
 - Multiplicative recurrence: within a chain of bins, E~_{k+1} = E~_k * F
   with F = exp(dw*2u/sig'^2), so each bin costs ONE VectorE/GpSimd multiply
   instead of an arg-build + ScalarE exp.  Chains are re-anchored with a
   fresh exp every few bins so fp32 underflow at chain starts cannot
   corrupt later bins (error bound ~e^{-87+span} << tolerance).  The
   per-bin scalar exp(-(w_k^2-w_anchor^2)/sig'^2) folds into M's rows.
 - z-reduction fused with the frame matrix: per bin one TensorE matmul
   with stationary [z=96, v=64] = M~[k,:] replicated (fp32r: 1 cycle/row
   since the moving fp32r operand is 384 wide), accumulating the final
   [v=64, (i,j)=384] cube directly in PSUM.  No per-i-row matmuls, no
   second downsample matmul.
 - j-downsample as two strided scalar_tensor_tensor stencil ops reading
   PSUM directly (3*a+b form, the 0.25 folds into M).
 - column-split pipelining (halves) so the second half's prep overlaps the
   first half's KDE chains.

Fallback: for degenerate runtime scalars (tiny sigma, wild velocity grids)
the same machinery runs in "direct" mode: w = velocity grid, sig' = sig,
M = norm*I, chains of length 1 (anchor-only) -- mathematically exact.

Sharding: 32 needed i-rows split 4-per-core across 8 cores; per-core device
layout [z=96 partitions, (i=4)x(j=96)=384 free].
"""

import math

import numpy as np

import concourse.bacc as bacc
import concourse.bass as bass
import concourse.mybir as mybir
import concourse.tile as tile
from concourse.bass_utils import run_bass_kernel_spmd

G = 96
NV = 64
N_CORES = 8
OUT_I = 32
ROWS_PER_CORE = OUT_I // N_CORES   # 4
PX = ROWS_PER_CORE * G             # 384
OUT_J = 64

F32 = mybir.dt.float32
F32R = mybir.dt.float32r
AF = mybir.ActivationFunctionType
OP = mybir.AluOpType

LAST_EXEC_NS = None

# tuning knobs
NSPLIT = 2            # column pipelining splits of the 384 free dim
CHAIN = 6             # bins per anchor chain
VG_PATTERN = None     # optional explicit engine pattern for KDE ops


def _plan_frame(si, sig2, vel):
    """Host-side: choose mode, w-grid, chain anchors and the M~ matrix.

    Returns (w, anchors, M) where M is (NK, NV) float32 with all scalar
    folds applied (norm, 0.25 stencil fold, per-chain recentering), or the
    direct-mode equivalent (w = vel, anchors = every bin, M = diag-ish).
    """
    sig = math.sqrt(sig2)
    norm = 1.0 / math.sqrt(2.0 * math.pi * sig2)
    umax = 200.0 * abs(si)
    sp2 = sig2 / 2.0
    sp = sig / math.sqrt(2.0)
    dw = 0.9 * sp
    span_lo = -umax - 1.0 * sp
    span_hi = umax + 1.0 * sp
    nk = int(math.ceil((span_hi - span_lo) / dw)) + 1
    nk = max(nk, 4)

    mode = "frame"
    if nk > 40:
        mode = "direct"
    else:
        w = np.linspace(span_lo, span_hi, nk)
        ddw = float(w[1] - w[0]) if nk > 1 else 0.0
        # F = exp(ddw*2u/sp2) must stay finite, and the per-chain recentered
        # running product bounded by e^{(w_k^2-w_a^2)/sp2} < e^75
        if ddw * 2.0 * umax / sp2 > 60.0:
            mode = "direct"
        wsq = w * w
        for a in range(0, nk, CHAIN):
            last = min(a + CHAIN, nk) - 1
            if abs(wsq[last] - wsq[a]) / sp2 > 75.0:
                mode = "direct"
                break

    velf = vel.astype(np.float64)
    if mode == "direct":
        w = velf.copy()
        anchors = list(range(len(w)))
        M = np.eye(len(w), NV) * norm
        return mode, w, anchors, M.astype(np.float64), sig2

    # least-squares fit of the 64 target kernels in the frame
    uu = max(umax, 2.0 * sp)
    us = np.linspace(-uu, uu, 4001)
    Phi = np.exp(-((us[:, None] - w[None, :]) ** 2) / sp2)
    T = np.exp(-((velf[None, :] - us[:, None]) ** 2) / sig2)
    M = np.linalg.solve(Phi.T @ Phi + 1e-8 * np.eye(nk), Phi.T @ T)
    M *= norm
    anchors = list(range(0, nk, CHAIN))
    # recentering fold: row k of chain anchored at a: M~ = M*exp(-(w_k^2-w_a^2)/sp2)
    for a in anchors:
        for k in range(a, min(a + CHAIN, nk)):
            M[k, :] *= math.exp(-(w[k] ** 2 - w[a] ** 2) / sp2)
    return mode, w, anchors, M, sp2


def _build_program(ci, si, cr, sr, sig2, vel):
    vel = np.asarray(vel, np.float32).reshape(-1)
    mode, w, anchors, M, sp2 = _plan_frame(si, sig2, vel)
    nk = len(w)
    anchor_of = {}
    for a in anchors:
        for k in range(a, min(a + CHAIN, nk) if mode == "frame" else a + 1):
            anchor_of[k] = a

    nc = bacc.Bacc("TRN2")

    xs = nc.dram_tensor("xs", [G, PX], F32, kind="ExternalInput")
    ys = nc.dram_tensor("ys", [G, PX], F32, kind="ExternalInput")
    zs = nc.dram_tensor("zs", [G, PX], F32, kind="ExternalInput")
    # M~ replicated down the z-partitions: [96, nk*64]
    ms = nc.dram_tensor("ms", [G, nk * NV], F32R, kind="ExternalInput")
    # per-anchor exp biases -w_a^2/sp2 (+ trailing Ln clamp), replicated
    na = len(anchors)
    bc = nc.dram_tensor("bc", [G, na + 1], F32, kind="ExternalInput")
    out = nc.dram_tensor("out", [NV, PX // 3 * 2], F32, kind="ExternalOutput")

    usc = -200.0 * si              # u = usc * cr-folded * tanh(r/2)*rx'/r
    spv = float(sp2)

    with tile.TileContext(nc) as tc:
        with (
            tc.tile_pool(name="io", bufs=1) as io,
            tc.tile_pool(name="prep", bufs=1) as prep,
            tc.tile_pool(name="kde", bufs=2) as kde,
            tc.tile_pool(name="psum", bufs=1, space="PSUM") as psum,
        ):
            xt = io.tile([G, PX], F32, tag="xt")
            yt = io.tile([G, PX], F32, tag="yt")
            zt = io.tile([G, PX], F32, tag="zt")
            nc.sync.dma_start(out=xt[:], in_=xs[:])
            nc.sync.dma_start(out=yt[:], in_=ys[:])
            nc.sync.dma_start(out=zt[:], in_=zs[:])
            mst = io.tile([G, nk * NV], F32R, tag="mst")
            nc.sync.dma_start(out=mst[:], in_=ms[:])
            bct = io.tile([G, na + 1], F32, tag="bct")
            nc.sync.dma_start(out=bct[:], in_=bc[:])
            aidx = {a: i for i, a in enumerate(anchors)}

            def vtile(name):
                return prep.tile([G, PX], F32, tag=name, name=name)

            t0 = vtile("t0")
            rxp = vtile("rxp")
            ryp = vtile("ryp")
            rzp = vtile("rzp")
            sqx = vtile("sqx")
            sqy = vtile("sqy")
            q = vtile("q")
            lnq = vtile("lnq")
            r = vtile("r")
            er = vtile("er")
            ed = vtile("ed")
            rec = vtile("rec")
            t1 = vtile("t1")
            u0f = vtile("u0f")
            u0 = vtile("u0")
            az = vtile("az")
            h1 = vtile("h1")
            ssq = vtile("ssq")
            A2 = vtile("A2")
            Ft = vtile("Ft") if mode == "frame" else None

            cube = psum.tile([NV, PX], F32)
            out_sb = io.tile([NV, PX // 3 * 2], F32, tag="out_sb")

            nsp = NSPLIT
            cols = [(h * PX // nsp, (h + 1) * PX // nsp) for h in range(nsp)]

            # ddw for F; in direct mode chains have length 1 and F is unused
            ddw = float(w[1] - w[0]) if (mode == "frame" and nk > 1) else 0.0

            mm_emitted = 0

            def emit_matmul(k, et):
                nonlocal mm_emitted
                nc.tensor.matmul(
                    cube[:],
                    mst[:, k * NV:(k + 1) * NV],
                    et[:],
                    start=(mm_emitted == 0), stop=(mm_emitted == nk - 1),
                    skip_group_check=True,
                )
                mm_emitted += 1

            # u = uk*u0 with uk = usc*cr ; w-term scalar on u0 is
            # w*2*uk/sp2 ; F = exp(ddw*2*uk/sp2 * u0)
            uk = usc * cr
            V, Gp, S = nc.vector, nc.gpsimd, nc.scalar
            for h, (c0, c1) in enumerate(cols):
                sl = slice(c0, c1)
                V.scalar_tensor_tensor(
                    t0[:, sl], xt[:, sl], sr / cr, yt[:, sl], OP.mult, OP.add)
                V.scalar_tensor_tensor(
                    rxp[:, sl], yt[:, sl], -sr / cr, xt[:, sl],
                    OP.mult, OP.add)
                V.scalar_tensor_tensor(
                    ryp[:, sl], zt[:, sl], -si / (ci * cr), t0[:, sl],
                    OP.mult, OP.add)
                V.scalar_tensor_tensor(
                    rzp[:, sl], zt[:, sl], ci / (si * cr) if si != 0 else 0.0,
                    t0[:, sl], OP.mult, OP.add)
                V.scalar_tensor_tensor(
                    sqx[:, sl], rxp[:, sl], cr * cr, rxp[:, sl],
                    OP.mult, OP.mult)
                Gp.tensor_mul(sqy[:, sl], ryp[:, sl], ryp[:, sl])
                V.scalar_tensor_tensor(
                    q[:, sl], sqy[:, sl], (ci * cr) ** 2, sqx[:, sl],
                    OP.mult, OP.add)
                S.activation(lnq[:, sl], q[:, sl], AF.Ln,
                             bias=bct[0:G, na:na + 1])
                S.activation(r[:, sl], lnq[:, sl], AF.Exp, scale=0.5)
                S.activation(er[:, sl], r[:, sl], AF.Exp)
                # den = (er+1)*r ; u0 = rxp*(er-1)/den ; u = uk*u0
                V.scalar_tensor_tensor(
                    ed[:, sl], er[:, sl], 1.0, r[:, sl], OP.add, OP.mult)
                V.reciprocal(rec[:, sl], ed[:, sl])
                Gp.tensor_mul(t1[:, sl], er[:, sl], rxp[:, sl])
                V.scalar_tensor_tensor(
                    u0f[:, sl], rxp[:, sl], -1.0, t1[:, sl], OP.mult, OP.add)
                V.scalar_tensor_tensor(
                    u0[:, sl], u0f[:, sl], 1.0, rec[:, sl], OP.mult, OP.mult)
                # A2 = -(h1)/3 - ssq ; h1 = 6|si*cr||rzp| + r
                S.activation(az[:, sl], rzp[:, sl], AF.Abs,
                             scale=6.0 * abs(si * cr))
                Gp.tensor_add(h1[:, sl], az[:, sl], r[:, sl])
                V.scalar_tensor_tensor(
                    ssq[:, sl], u0[:, sl], uk * uk / spv, u0[:, sl],
                    OP.mult, OP.mult)
                V.scalar_tensor_tensor(
                    A2[:, sl], h1[:, sl], -1.0 / 3.0, ssq[:, sl],
                    OP.mult, OP.subtract)
                if mode == "frame":
                    S.activation(Ft[:, sl], u0[:, sl], AF.Exp,
                                 scale=ddw * 2.0 * uk / spv)

            # --- KDE: anchors + multiplicative chains, interleaved waves ---
            echain = {}   # chain anchor -> current E tile
            vgi = 0
            maxlen = CHAIN if mode == "frame" else 1
            for step in range(maxlen):
                for a in anchors:
                    k = a + step
                    if k >= nk or anchor_of.get(k) != a:
                        continue
                    et = kde.tile([G, PX], F32R, tag="e%d" % (a % (2 * CHAIN)),
                                  bufs=4)
                    if step == 0:
                        # arg = A2 + w_a*(2uk/sp2)*u0 ; exp bias = -w_a^2/sp2
                        arg = kde.tile([G, PX], F32, tag="arg", bufs=4)
                        for c0, c1 in cols:
                            sl = slice(c0, c1)
                            nc.vector.scalar_tensor_tensor(
                                arg[:, sl], u0[:, sl],
                                float(w[a]) * 2.0 * uk / spv,
                                A2[:, sl], OP.mult, OP.add)
                            nc.scalar.activation(
                                et[:, sl], arg[:, sl], AF.Exp,
                                bias=bct[0:G, aidx[a]:aidx[a] + 1])
                    else:
                        # chain multiply (tensor_tensor works on V and G)
                        ep = echain[a]
                        for c0, c1 in cols:
                            sl = slice(c0, c1)
                            eng = nc.gpsimd if (vgi % 2 == 1) else nc.vector
                            vgi += 1
                            eng.tensor_mul(et[:, sl], ep[:, sl], Ft[:, sl])
                    echain[a] = et
                    emit_matmul(k, et)

            # --- j-downsample stencil straight out of PSUM ---
            # even jj=2m: 0.75*c[3m] + 0.25*c[3m+1] = (3*c[3m]+c[3m+1])/4
            # odd  jj=2m+1: (3*c[3m+2]+c[3m+1])/4 ; the /4 is folded into M
            cube_sb = io.tile([NV, PX], F32, tag="cube_sb")
            nc.vector.tensor_copy(cube_sb[:], cube[:])
            nc.vector.scalar_tensor_tensor(
                out_sb[:, 0:PX // 3 * 2:2], cube_sb[:, 0:PX:3], 3.0,
                cube_sb[:, 1:PX:3], OP.mult, OP.add)
            nc.vector.scalar_tensor_tensor(
                out_sb[:, 1:PX // 3 * 2:2], cube_sb[:, 2:PX:3], 3.0,
                cube_sb[:, 1:PX:3], OP.mult, OP.add)
            nc.sync.dma_start(out=out[:], in_=out_sb[:])

    return nc, mode, w, anchors, M, sp2


def _host_constants(M, nk):
    # M~ scaled by stencil fold 0.25, replicated down 96 partitions
    Mr = (0.25 * M).astype(np.float32)            # (nk, 64)
    return np.ascontiguousarray(
        np.tile(Mr.reshape(1, nk * NV), (G, 1)))


def kernel(**inputs):
    inc = float(np.asarray(inputs["inclination"]).reshape(-1)[0])
    rot = float(np.asarray(inputs["sky_rot"]).reshape(-1)[0])
    lb = float(np.asarray(inputs["line_broadening"]).reshape(-1)[0])
    vel = np.asarray(inputs["velocity_grid"], np.float32).reshape(-1)
    X = np.asarray(inputs["Xgrid"], np.float32)
    Y = np.asarray(inputs["Ygrid"], np.float32)
    Z = np.asarray(inputs["Zgrid"], np.float32)

    ci, si = math.cos(inc), math.sin(inc)
    cr, sr = math.cos(rot), math.sin(rot)
    sig2 = float(np.float32(lb) * np.float32(lb))
    if not (sig2 > 0.0) or not math.isfinite(sig2):
        sig2 = 1e-30

    nc, mode, w, anchors, M, sp2 = _build_program(ci, si, cr, sr, sig2, vel)
    nc.finalize()

    msv = _host_constants(M, len(w))
    bcv = np.ascontiguousarray(np.tile(
        np.asarray([-w[a] * w[a] / sp2 for a in anchors] + [1e-30],
                   np.float32).reshape(1, -1), (G, 1)))

    in_maps = []
    for c in range(N_CORES):
        rows = [3 * k + 1 for k in range(ROWS_PER_CORE * c,
                                         ROWS_PER_CORE * (c + 1))]
        def shard(a):
            s = a[rows]                              # (4, 96, 96) = (i, j, z)
            s = s.transpose(2, 0, 1).reshape(G, PX)  # [z, i*96+j]
            return np.ascontiguousarray(s)
        in_maps.append({"xs": shard(X), "ys": shard(Y), "zs": shard(Z),
                        "ms": msv, "bc": bcv})

    res = run_bass_kernel_spmd(nc, in_maps, core_ids=list(range(N_CORES)))
    global LAST_EXEC_NS
    LAST_EXEC_NS = res.exec_time_ns

    parts = []
    for c in range(N_CORES):
        o = res.results[c]["out"]                    # (64, 256) = [v,(i,m,p)]
        parts.append(o.reshape(NV, ROWS_PER_CORE, OUT_J)
                      .transpose(1, 2, 0))           # (4, 64jj, 64v)
    return np.concatenate(parts, axis=0).astype(np.float32)  # (32, 64, 64)
